# revision 20
# baseline (speedup 1.0000x reference)
"""ConvQRNN Trainium2 kernel.

Strategy (8 NeuronCores, spatial H-sharding, 8 rows/core):
  - Conv3d(k=(2,3,3), CIN=3 -> 256) lowered to matmul: host builds a fp16
    im2col with K=56 rows (54 taps + ones row carrying the conv bias + one
    zero pad row).
  - Scan layout: [128, 1024] fp16 per step with partition = (b//2)*64 + ch,
    free = (b%2)*512 + h*64 + w.  The two column halves (q = b%2) are
    independent scans interleaved as two dependency chains.
  - C state lives at a FIXED address in ctb = [C0|tg0|C1|tg1]; tanh(g) is
    written next to C so the fused [s_f|s_i]*[C|tanh(g)] multiply reads one
    contiguous operand.  All fp16 DVE ops keep src0/src1/dst congruent
    mod 4KB, which HW requires for the 2x DVE mode.
  - o-gate: DVE writes Wco*C into the o PSUM bank, then the o-gate conv
    matmul ACCUMULATES on top (start=False), so sigmoid reads the finished
    pre-activation straight from PSUM.  No separate o-gate add.
  - Off-chain ops (tanh g, tanh C, sigmoid o, H-mul) are merged across the
    two halves into single 1024-col ops and deferred one step so they never
    stall the recurrence chain.
"""

import os

import numpy as np

B, CIN, T, H, W = 4, 3, 32, 64, 64
COUT = 64
NC = 8
HS = H // NC
K = 56
PIX = B * HS * W          # 2048
F = PIX // 2              # 1024
FH = F // 2               # 512
KAPPA = 8
NW = T // KAPPA

f16 = np.float16

_CACHE = {}
LAST_RESULTS = {}


def _host_prep(X, Wconv, bconv, W_ci, W_cf, W_co):
    X = np.ascontiguousarray(np.asarray(X, np.float32))
    Wconv = np.asarray(Wconv, np.float32)
    bconv = np.asarray(bconv, np.float32)
    Xp = np.pad(X, ((0, 0), (0, 0), (1, 0), (1, 1), (1, 1)))

    im2col = np.zeros((NC, K, T, PIX), f16)
    for c in range(NC):
        for cin in range(CIN):
            for dt in range(2):
                for dh in range(3):
                    for dw in range(3):
                        k = ((cin * 2 + dt) * 3 + dh) * 3 + dw
                        blk = Xp[:, cin, dt:dt + T,
                                 8 * c + dh:8 * c + dh + HS, dw:dw + W]
                        blk = blk.reshape(2, 2, T, HS, W).transpose(2, 0, 1, 3, 4)
                        im2col[c, k] = blk.reshape(T, PIX).astype(f16)
        im2col[c, 54] = 1.0

    # kernel gate order: (f, i, g, o)
    gate_order = (1, 0, 2, 3)
    lhsT = np.zeros((4, K, 128), f16)
    Wr = Wconv.reshape(4, COUT, CIN, 2, 3, 3)
    for gi, g in enumerate(gate_order):
        wk = Wr[g].transpose(1, 2, 3, 4, 0).reshape(54, COUT).astype(f16)
        lhsT[gi, :54, :64] = wk
        lhsT[gi, :54, 64:] = wk
        lhsT[gi, 54, :64] = bconv[g * 64:(g + 1) * 64].astype(f16)
        lhsT[gi, 54, 64:] = bconv[g * 64:(g + 1) * 64].astype(f16)

    # peep[c]: [128, 2048] = [Wcf | Wci | Wco | Wco], rows duplicated over
    # the two row-halves (both hold the same 64 channels).
    peep = np.zeros((NC, 128, 4 * FH), f16)
    for c in range(NC):
        for i, Wc in enumerate((W_cf, W_ci, W_co, W_co)):
            sl = np.asarray(Wc, np.float32)[:, 8 * c:8 * c + HS, :]
            sl = sl.reshape(64, FH).astype(f16)
            peep[c, :64, i * FH:(i + 1) * FH] = sl
            peep[c, 64:, i * FH:(i + 1) * FH] = sl
    return im2col, lhsT, peep


def _build_nc():
    import concourse.bacc as bacc
    import concourse.mybir as mybir
    from concourse.tile import TileContext

    fp16 = mybir.dt.float16
    fp32 = mybir.dt.float32
    AF = mybir.ActivationFunctionType

    nc = bacc.Bacc(None, target_bir_lowering=False)

    im2col_d = nc.dram_tensor("im2col", [K, T, PIX], fp16, kind="ExternalInput")
    lhsT_d = nc.dram_tensor("lhsT", [4, K, 128], fp16, kind="ExternalInput")
    peep_d = nc.dram_tensor("peep", [128, 4 * FH], fp16, kind="ExternalInput")
    out_d = nc.dram_tensor("out", [T, 128, F], fp16, kind="ExternalOutput")

    with TileContext(nc) as tc:
        with (
            tc.tile_pool(name="const", bufs=1) as constp,
            tc.tile_pool(name="al", bufs=1) as alp,
            tc.tile_pool(name="rhs", bufs=3) as rhsp,
            tc.tile_pool(name="psum", bufs=1, space="PSUM") as psump,
        ):
            wcif = constp.tile([128, F], fp16)       # [Wcf | Wci]
            wcoD = constp.tile([128, F], fp16)       # [Wco | Wco]
            nc.sync.dma_start(out=wcif[:], in_=peep_d[:, 0:F])
            nc.sync.dma_start(out=wcoD[:], in_=peep_d[:, F:2 * F])
            lhsT_sb = constp.tile([K, 4 * 128], fp16)
            nc.sync.dma_start(
                out=lhsT_sb[:].rearrange("k (g m) -> k g m", g=4),
                in_=lhsT_d[:].rearrange("g k m -> k g m"),
            )

            # aligned arena: every tile a 4KB multiple so all bases (and
            # equal-offset slices) stay congruent mod 4KB -> DVE 2x mode
            vv = alp.tile([128, 2 * F], fp16)        # [vf0|vi0|vf1|vi1]
            ss = alp.tile([128, 2 * F], fp16)        # sigmoid outputs
            ctb = alp.tile([128, 2 * F], fp16)       # [C0|tg0|C1|tg1]
            soh = alp.tile([128, KAPPA * F], fp16)   # sigmoid(a_o) slots
            tch = alp.tile([128, KAPPA * F], fp16)   # tanh(C) slots
            h8 = [alp.tile([128, KAPPA * F], fp16, name=f"h8{p}")
                  for p in range(2)]

            e_if = psump.tile([128, 2 * F], fp32)    # [f0|i0|f1|i1]
            e_g = psump.tile([128, F], fp32)         # [g0|g1]
            e_o = psump.tile([128, F], fp32)         # [o0|o1]

            nc.vector.memset(ctb[:, 0:FH], 0.0)
            nc.vector.memset(ctb[:, F:F + FH], 0.0)

            def cslice(q):
                return ctb[:, q * F:q * F + FH]

            def c2seg():
                # [C0 | C1] as a 2-segment strided AP
                return ctb[:].rearrange("p (s f) -> p s f", s=2)[:, :, 0:FH]

            def tg2seg():
                # [tg0 | tg1]
                return ctb[:].rearrange("p (s f) -> p s f", s=2)[:, :, FH:F]

            for q in range(2):
                nc.vector.tensor_mul(
                    out=e_if[:, q * F:(q + 1) * F]
                    .rearrange("p (o f) -> p o f", o=2),
                    in0=wcif[:].rearrange("p (o f) -> p o f", o=2),
                    in1=cslice(q).rearrange("p (o f) -> p o f", o=1)
                    .broadcast_to([128, 2, FH]),
                )

            rhs_t = {}
            for t in range(T):
                j = t % KAPPA
                w = t // KAPPA
                par = w % 2

                rhs = rhsp.tile([K, PIX], fp16)
                nc.sync.dma_start(out=rhs[:], in_=im2col_d[:, t, :])
                rhs_t[t] = rhs

                pj = (t - 1) % KAPPA
                ppar = ((t - 1) // KAPPA) % 2

                # ---- DVE: Wco*C(t-1) into the o banks (vif(t) was emitted
                #      at the end of iteration t-1, right after Cn(t-1)) ----
                if t > 0:
                    nc.vector.tensor_mul(
                        out=e_o[:].rearrange("p (s f) -> p s f", s=2),
                        in0=wcoD[:].rearrange("p (s f) -> p s f", s=2),
                        in1=c2seg(),
                    )

                # ---- PE: g first (dep-free), then f/i q0, f/i q1, then the
                #      previous step's o-gate accumulate ----
                def mm(gi, hf, q, which_rhs, start):
                    lw = lhsT_sb[:, gi * 128 + 64 * hf:gi * 128 + 64 * hf + 64]
                    if gi < 2:
                        tgt = e_if[64 * hf:64 * hf + 64,
                                   q * F + gi * FH:q * F + (gi + 1) * FH]
                    elif gi == 2:
                        tgt = e_g[64 * hf:64 * hf + 64, q * FH:(q + 1) * FH]
                    else:
                        tgt = e_o[64 * hf:64 * hf + 64, q * FH:(q + 1) * FH]
                    b = 2 * hf + q
                    nc.tensor.matmul(
                        tgt, lw, which_rhs[:, b * FH:(b + 1) * FH],
                        start=start, stop=True, tile_position=(0, 64 * hf),
                    )

                for hf in range(2):
                    for q in range(2):
                        mm(2, hf, q, rhs, True)
                for q in range(2):
                    for gi in (0, 1):
                        for hf in range(2):
                            mm(gi, hf, q, rhs, False)
                if t > 0:
                    for hf in range(2):
                        for q in range(2):
                            mm(3, hf, q, rhs_t[t - 1], False)
                    del rhs_t[t - 1]

                # ---- ACT: tc(t-1), tg(t), s0, s1, so(t-1) ----
                if t > 0:
                    nc.scalar.activation(
                        tch[:, pj * F:(pj + 1) * F], c2seg(), AF.Tanh,
                    )
                nc.scalar.activation(tg2seg(), e_g[:], AF.Tanh)
                for q in range(2):
                    nc.scalar.activation(
                        ss[:, q * F:(q + 1) * F],
                        e_if[:, q * F:(q + 1) * F], AF.Sigmoid,
                    )
                if t > 0:
                    nc.scalar.activation(
                        soh[:, pj * F:(pj + 1) * F], e_o[:], AF.Sigmoid
                    )

                # ---- DVE: pp/Cn chains; vif(t+1) follows Cn_q(t) directly
                #      so the next cycle starts with zero DVE queue delay ----
                for q in range(2):
                    nc.vector.tensor_mul(
                        out=vv[:, q * F:(q + 1) * F],
                        in0=ss[:, q * F:(q + 1) * F],
                        in1=ctb[:, q * F:(q + 1) * F],
                    )
                    nc.vector.tensor_add(
                        out=cslice(q),
                        in0=vv[:, q * F:q * F + FH],
                        in1=vv[:, q * F + FH:(q + 1) * F],
                    )
                    if t + 1 < T:
                        nc.vector.tensor_mul(
                            out=e_if[:, q * F:(q + 1) * F]
                            .rearrange("p (o f) -> p o f", o=2),
                            in0=wcif[:].rearrange("p (o f) -> p o f", o=2),
                            in1=cslice(q).rearrange("p (o f) -> p o f", o=1)
                            .broadcast_to([128, 2, FH]),
                        )
                if t > 0:
                    nc.vector.tensor_mul(
                        out=h8[ppar][:, pj * F:(pj + 1) * F],
                        in0=soh[:, pj * F:(pj + 1) * F],
                        in1=tch[:, pj * F:(pj + 1) * F],
                    )
                    if pj == KAPPA - 1:
                        k0 = ((t - 1) // KAPPA) * KAPPA
                        for q in range(2):
                            nc.sync.dma_start(
                                out=out_d[k0:k0 + KAPPA, :, q * FH:(q + 1) * FH]
                                .rearrange("t p f -> p t f"),
                                in_=h8[ppar][:]
                                .rearrange("p (t s f) -> p t s f",
                                           t=KAPPA, s=2)[:, :, q, :],
                            )

            # ---- epilogue: last step's o-gate + tail ----
            t = T - 1
            pj = t % KAPPA
            ppar = (t // KAPPA) % 2
            nc.vector.tensor_mul(
                out=e_o[:].rearrange("p (s f) -> p s f", s=2),
                in0=wcoD[:].rearrange("p (s f) -> p s f", s=2),
                in1=c2seg(),
            )
            for hf in range(2):
                lw = lhsT_sb[:, 3 * 128 + 64 * hf:3 * 128 + 64 * hf + 64]
                for q in range(2):
                    b = 2 * hf + q
                    nc.tensor.matmul(
                        e_o[64 * hf:64 * hf + 64, q * FH:(q + 1) * FH],
                        lw,
                        rhs_t[t][:, b * FH:(b + 1) * FH],
                        start=False, stop=True,
                        tile_position=(0, 64 * hf),
                    )
            nc.scalar.activation(tch[:, pj * F:(pj + 1) * F], c2seg(), AF.Tanh)
            nc.scalar.activation(soh[:, pj * F:(pj + 1) * F], e_o[:], AF.Sigmoid)
            nc.vector.tensor_mul(
                out=h8[ppar][:, pj * F:(pj + 1) * F],
                in0=soh[:, pj * F:(pj + 1) * F],
                in1=tch[:, pj * F:(pj + 1) * F],
            )
            k0 = (NW - 1) * KAPPA
            for q in range(2):
                nc.sync.dma_start(
                    out=out_d[k0:k0 + KAPPA, :, q * FH:(q + 1) * FH]
                    .rearrange("t p f -> p t f"),
                    in_=h8[ppar][:]
                    .rearrange("p (t s f) -> p t s f", t=KAPPA, s=2)[:, :, q, :],
                )

    nc.compile()
    return nc


def _get_nc():
    if "nc" not in _CACHE:
        _CACHE["nc"] = _build_nc()
    return _CACHE["nc"]


def kernel(X, Wconv, bconv, W_ci, W_cf, W_co):
    from concourse.bass_utils import run_bass_kernel_spmd

    im2col, lhsT, peep = _host_prep(X, Wconv, bconv, W_ci, W_cf, W_co)
    nc = _get_nc()
    in_maps = [
        {"im2col": im2col[c], "lhsT": lhsT, "peep": peep[c]} for c in range(NC)
    ]
    trace = bool(os.environ.get("QRNN_TRACE"))
    res = run_bass_kernel_spmd(
        nc, in_maps, core_ids=list(range(NC)), trace=trace
    )
    LAST_RESULTS["exec_time_ns"] = getattr(res, "exec_time_ns", None)

    O = np.empty((B, COUT, T, H, W), np.float32)
    for c in range(NC):
        o = np.asarray(res.results[c]["out"], f16).astype(np.float32)
        o = o.reshape(T, 2, 64, 2, HS, W).transpose(1, 3, 2, 0, 4, 5)
        O[:, :, :, 8 * c:8 * c + HS, :] = o.reshape(B, COUT, T, HS, W)
    return O
